# revision 60
# baseline (speedup 1.0000x reference)
"""MetaSR super-resolution Trainium2 kernel.

Structure exploited: out_h=out_w=256 with H=W=64 LR grid means the scale
factor is exactly 4, so the nearest-neighbor gather index is iy=oy//4,
ix=ox//4 and the per-query MLP input collapses to 16 distinct subpixel
phases [dy/4, dx/4, 0.25].  The whole model becomes a 3x3 conv with 64
input / 48 output channels (3 RGB x 16 phases) + pixel shuffle, whose
48x576 weight predw = relu([16,3] @ w1 + b1) @ w2 + b2 is a tiny
16-phase MLP evaluated host-side (14 MFLOP of the model's 240 MFLOP;
the 226 MFLOP conv runs on device).

Sharding: data-parallel over LR rows (8 rows per core, 10-row halo band),
conv weights replicated.

The conv contraction (K = 9 taps x 64 ch = 576) is chunked K=128 by
pairing taps.  Each core holds the zero-padded band twice in a
128-partition tile at free-dim offsets that differ by the two taps'
shift delta, so one K=128 matmul consumes two taps without
materializing the unfolded tensor:
  band free index = r*66 + x  (66-wide zero-padded rows), tap (ki,kj)
  shift = ki*66 + kj.
Three K=128 chunks pair the delta-1 taps; the three leftover taps run
as K=64 chunks on the band@+1 half.  Everything (band + weights) rides
ONE fat DMA (1902B rows, few descriptors) — a single semaphore gates
the whole conv.

Each chunk issues TWO 256-col matmuls (band rows 0-3 -> PSUM
partitions 0-47, rows 4-7 -> 64-111) sharing one stationary load: the
two PE output row groups (h0/h64) dual-issue, running the pair
concurrently — the conv takes ~1.5us vs ~2.5us for 512-col single
matmuls.  The wider [128, 256] result also doubles the output cast's
DVE lanes and spreads the output DMA across all 16 SDMA engines.

Band and weights are bf16 (PSUM accumulates fp32): halves DMA traffic.
The output is written back as bf16 (rel err ~3.3e-3 vs the 2e-2 gate)
and widened host-side.

All DMAs ride a single HWDGE queue (SP): both HWDGE queues share the
same 16 SDMA engines, so a second queue adds no bandwidth, and keeping
blob_b on the same queue guarantees its packets never interleave ahead
of blob_a's (which gates the conv start).  The unused SWDGE
(qPoolDynamic) and ACT queue declarations are stripped from the module
before compile (fewer declared rings; the runtime's fixed ~7.6us
semaphore-clear postamble is unaffected by this, measured).

A run of dummy matmuls (zero scratch, overwritten by the first conv
accumulation via start=True) warms the PE HAM clock gate while the
DMAs land.
"""

import os

import ml_dtypes
import numpy as np

try:
    import concourse.bass as bass
except ImportError:  # fall back to the repo checkout
    import sys
    sys.path.insert(0, "/opt/trn_rl_repo")
    import concourse.bass as bass
import concourse.mybir as mybir
import concourse.tile as tile
from concourse import bacc
from concourse.bass_utils import run_bass_kernel_spmd

F32 = mybir.dt.float32
F32R = mybir.dt.float32r
BF16 = mybir.dt.bfloat16
N_CORES = 8
ROWS_PER_CORE = 8          # LR rows per core
BAND_ROWS = ROWS_PER_CORE + 2
NPOS = ROWS_PER_CORE * 64  # 512 LR positions per core

# Taps t = ki*3+kj have band shift ki*66+kj.  band1 (in blob_a) holds the
# zero-padded band twice: p0-63 = band@+1 (a chunk at rhs offset roff
# sees shift roff-1), p64-127 = band@0 (shift roff).  K=128 chunks pair
# the delta-1 taps; taps 2/3/8 run K=64 on the p0-63 half (shift roff-1).
#   (band_tensor_idx, rhs_offset, K, taps, p_base)
ORDER = [
    (0, 1, 128, (0, 1), 0),     # shifts 0,1
    (0, 68, 128, (4, 5), 0),    # shifts 67,68
    (0, 133, 128, (6, 7), 0),   # shifts 132,133
    (0, 3, 64, (2,), 0),        # shift 2
    (0, 67, 64, (3,), 0),       # shift 66
    (0, 135, 64, (8,), 0),      # shift 134
]
COLS_B1 = 663  # 661 + pad cols so the tap-8 chunk's AP (135 + 8*66) fits
COLS_B2 = 528  # unused (kept for reference)
COLS_W = 6 * 48
COLS_A = COLS_B1 + COLS_W  # band ++ W in one blob -> one fat DMA

N_WARMUP_MM = 5

USE_BF16 = os.environ.get("METASR_DTYPE", "bf16") == "bf16"
QUEUE_MODE = os.environ.get("METASR_QUEUES", "sp16")

_CACHE = {}


def _build_program(use_bf16, queue_mode, early_dma=False, trim_end=True):
    """Build + compile the single-core Bass program (same for all cores)."""
    nc = bacc.Bacc("TRN2", target_bir_lowering=False, debug=False)

    dt = BF16 if use_bf16 else F32R
    odt = BF16 if use_bf16 else F32
    blob_a_d = nc.dram_tensor("blob_a", [128, COLS_A], dt, kind="ExternalInput")
    out48 = nc.dram_tensor("out48", [128, NPOS // 2], odt, kind="ExternalOutput")

    single_q = queue_mode == "sp16"

    with tile.TileContext(nc) as tc:
        with (
            tc.tile_pool(name="blobs", bufs=1) as blobs,
            tc.tile_pool(name="work", bufs=1) as work,
            tc.tile_pool(name="opool", bufs=1) as opool,
            tc.tile_pool(name="ps_rgb", bufs=1, space="PSUM") as ps_rgb,
        ):
            blob_a = blobs.tile([128, COLS_A], dt, tag="blob_a")
            nc.sync.dma_start(blob_a[:, :], blob_a_d[:, :])
            band1 = blob_a[:, 0:COLS_B1]
            wtile = blob_a[:, COLS_B1:COLS_A]

            # The conv output lives as [128, 256]: band rows 0-3 (256
            # positions) land in PSUM partitions 0-47, rows 4-7 in 64-111
            # (PSUM matmul base must be 0/32/64; rows 48-63/112-127 are
            # junk the host ignores).  Each chunk issues two 256-col
            # matmuls sharing one stationary load; the wider output
            # doubles the cast's DVE lanes and spreads the output DMA
            # across all 16 SDMA engines.
            hp = NPOS // 2
            rgb_ps = ps_rgb.tile([128, hp], F32, tag="rgb")
            warm = work.tile([128, NPOS], BF16, tag="warm")
            nc.vector.memset(warm[:, :], 0.0)
            nc.vector.memset(rgb_ps[:, :], 0.0)
            for i in range(N_WARMUP_MM):
                pbase = 64 * (i % 2)
                nc.tensor.matmul(
                    rgb_ps[pbase:pbase + 48, :], warm[:, 0:48], warm[:, 0:hp],
                    start=True, stop=True,
                )

            bands = [band1]
            last = len(ORDER) - 1
            for m, (bidx, roff, K, _taps, pb) in enumerate(ORDER):
                for s in range(2):
                    ro = roff + s * 4 * 66
                    rhs = bands[bidx][pb:pb + K, ro:ro + 4 * 66].rearrange(
                        "p (r c) -> p r c", c=66
                    )[:, :, 0:64]
                    nc.tensor.matmul(
                        rgb_ps[s * 64:s * 64 + 48, :],
                        wtile[pb:pb + K, m * 48:(m + 1) * 48], rhs,
                        start=(m == 0), stop=(m == last),
                    )

            # ---- write out: one cast copy + one DMA ----
            out_sb = opool.tile([128, hp], odt, tag="out")
            nc.vector.tensor_copy(out_sb[:, :], rgb_ps[:, :])
            nc.sync.dma_start(out48[:, :], out_sb[:, :])

    if single_q:
        # Both HWDGE queues share the 16 SDMA engines, and the runtime's
        # fixed postamble (semaphore-clear sweep) scales with declared
        # queue rings — keep only the SP HWDGE queue actually used.
        used = {"qSPDynamicHW"}
        nc.m.queues = [q for q in nc.m.queues if q.name in used]

    if early_dma:
        # Hoisting the blob_a DMA issue ahead of the framework's const-AP
        # memsets + barrier was tried and measured SLOWER (~16.7us vs
        # ~15.9us median): the transfer drains slower when it overlaps the
        # other cores' preamble phase, outweighing the ~0.7us earlier
        # issue.  Kept as a flag for reference; default off.
        main_blk, tile_blk = nc.m.functions[0].blocks[0], nc.m.functions[0].blocks[1]
        dma = next(i for i in tile_blk.instructions
                   if type(i).__name__ == "InstDMACopy")
        assert not (dma.sync_info and dma.sync_info.on_wait)
        tile_blk.instructions.remove(dma)
        main_blk.instructions.insert(1, dma)

    if trim_end:
        # The TileContext epilogue ends with TWO all-engine barriers; the
        # second is redundant (every semaphore is stable after the first,
        # which already follows SP's DMA-drain waits and the tile
        # RANGE_CLEAR).  Dropping it lets each engine flow into the
        # runtime's fixed semaphore-clear postamble one barrier earlier.
        eb = nc.m.functions[0].blocks[-1]
        names = [type(i).__name__ for i in eb.instructions]
        isa_idx = names.index("InstISA")  # the tile RANGE_CLEAR on Pool
        assert isa_idx > len(names) // 3, (names, isa_idx)
        del eb.instructions[isa_idx + 1:]

        # Letting PE and Activation ALSO skip this barrier (so their
        # postamble semaphore-clear slices overlap the conv/DMAs) was
        # tried and measured SLOWER (17.6 vs 15.7us): clearing the
        # walrus-range sems 7-104 while DMAs are in flight throttles the
        # queue machinery.  Keep all engines behind the DMA drain.

    nc.compile()
    return nc


def _round_f32r(x):
    """Round fp32 to the fp32r-representable set (bf16 hi + bf16 lo pair)."""
    hi = x.astype(ml_dtypes.bfloat16).astype(np.float32)
    lo = (x - hi).astype(ml_dtypes.bfloat16).astype(np.float32)
    return hi + lo


def _host_prep(feat, w1, b1, w2, b2, use_bf16):
    """Compute the 16-phase conv weights and pack per-core band blobs."""
    feat = np.ascontiguousarray(np.asarray(feat, dtype=np.float32))[0]  # [64,64,64]
    w1 = np.asarray(w1, dtype=np.float32)
    b1 = np.asarray(b1, dtype=np.float32)
    w2 = np.asarray(w2, dtype=np.float32)
    b2 = np.asarray(b2, dtype=np.float32)

    dydx = np.arange(16)
    mlpin = np.stack(
        [dydx // 4 / 4.0, dydx % 4 / 4.0, np.full(16, 0.25)], axis=1
    ).astype(np.float32)  # [16, 3]
    h = np.maximum(mlpin @ w1 + b1, 0.0).astype(np.float32)      # [16, 256]
    pw = (h @ w2 + b2).astype(np.float32).reshape(16, 64, 9, 3)  # [ph, c, t, o]

    wblob = np.zeros((128, COLS_W), dtype=np.float32)
    for m, (_bidx, _roff, _K, taps, pb) in enumerate(ORDER):
        for slot, t in enumerate(taps):
            # rows pb + slot*64 + c ; cols m*48 + o*16 + ph
            r0 = pb + slot * 64
            wblob[r0:r0 + 64, m * 48:(m + 1) * 48] = \
                pw[:, :, t, :].transpose(1, 2, 0).reshape(64, 48)

    featp = np.zeros((64, 66, 66), dtype=np.float32)
    featp[:, 1:65, 1:65] = feat

    if use_bf16:
        wblob = wblob.astype(ml_dtypes.bfloat16)
        featp = featp.astype(ml_dtypes.bfloat16)
    else:
        wblob = _round_f32r(wblob)
        featp = _round_f32r(featp)
    ndt = featp.dtype

    blobs_a = []
    for core in range(N_CORES):
        r0 = core * ROWS_PER_CORE
        band = featp[:, r0:r0 + BAND_ROWS, :].reshape(64, BAND_ROWS * 66)
        ab = np.zeros((128, COLS_A), dtype=ndt)
        ab[0:64, 1:661] = band
        ab[64:128, 0:660] = band
        ab[:, COLS_B1:COLS_A] = wblob
        blobs_a.append(ab)
    return blobs_a


def _assemble(per_core_out48):
    """[8 x [128, 256]] -> [1, 3, 256, 256].

    Rows 0-47 / 64-111 of a core's block hold band rows 0-3 / 4-7
    (rows 48-63 and 112-127 are junk).  Within a half, row j is
    (o = j//16, dy = (j%16)//4, dx = j%4); col is (r4, x); the LR row
    is core*8 + half*4 + r4.
    """
    full = np.stack([np.asarray(o, dtype=np.float32) for o in per_core_out48])
    full = full.reshape(8, 2, 64, 4, 64)[:, :, 0:48]  # [core, half, 48, r4, x]
    full = full.reshape(8, 2, 3, 4, 4, 4, 64)  # [core, half, o, dy, dx, r4, x]
    rgb = full.transpose(2, 0, 1, 5, 3, 6, 4).reshape(3, 256, 256)
    return np.ascontiguousarray(rgb)[None]


def get_program():
    key = ("nc", USE_BF16, QUEUE_MODE)
    if key not in _CACHE:
        _CACHE[key] = _build_program(USE_BF16, QUEUE_MODE)
    return _CACHE[key]


def run(feat, w1, b1, w2, b2, out_h, out_w, trace=False, **spmd_kwargs):
    assert int(out_h) == 256 and int(out_w) == 256
    nc = get_program()
    blobs_a = _host_prep(feat, w1, b1, w2, b2, USE_BF16)
    in_maps = [{"blob_a": blobs_a[core]} for core in range(N_CORES)]
    res = run_bass_kernel_spmd(
        nc, in_maps, core_ids=list(range(N_CORES)), trace=trace, **spmd_kwargs
    )
    out = _assemble([res.results[core]["out48"] for core in range(N_CORES)])
    return out, res


def kernel(feat, w1, b1, w2, b2, out_h, out_w):
    out, _ = run(feat, w1, b1, w2, b2, out_h, out_w, trace=False)
    return out


# revision 61
# speedup vs baseline: 1.0313x; 1.0313x over previous
"""MetaSR super-resolution Trainium2 kernel.

Structure exploited: out_h=out_w=256 with H=W=64 LR grid means the scale
factor is exactly 4, so the nearest-neighbor gather index is iy=oy//4,
ix=ox//4 and the per-query MLP input collapses to 16 distinct subpixel
phases [dy/4, dx/4, 0.25].  The whole model becomes a 3x3 conv with 64
input / 48 output channels (3 RGB x 16 phases) + pixel shuffle, whose
48x576 weight predw = relu([16,3] @ w1 + b1) @ w2 + b2 is a tiny
16-phase MLP evaluated host-side (14 MFLOP of the model's 240 MFLOP;
the 226 MFLOP conv runs on device).

Sharding: data-parallel over LR rows (8 rows per core, 10-row halo band),
conv weights replicated.

The conv contraction (K = 9 taps x 64 ch = 576) is chunked K=128 by
pairing taps.  Each core holds the zero-padded band twice in a
128-partition tile at free-dim offsets that differ by the two taps'
shift delta, so one K=128 matmul consumes two taps without
materializing the unfolded tensor:
  band free index = r*66 + x  (66-wide zero-padded rows), tap (ki,kj)
  shift = ki*66 + kj.
Three K=128 chunks pair the delta-1 taps; the three leftover taps run
as K=64 chunks on the band@+1 half.  Everything (band + weights) rides
ONE fat DMA (1902B rows, few descriptors) — a single semaphore gates
the whole conv.

Each chunk issues TWO 256-col matmuls (band rows 0-3 -> PSUM
partitions 0-47, rows 4-7 -> 64-111) sharing one stationary load: the
two PE output row groups (h0/h64) dual-issue, running the pair
concurrently — the conv takes ~1.5us vs ~2.5us for 512-col single
matmuls.  The wider [128, 256] result also doubles the output cast's
DVE lanes and spreads the output DMA across all 16 SDMA engines.

Band and weights are bf16 (PSUM accumulates fp32): halves DMA traffic.
The output is written back as bf16 (rel err ~3.3e-3 vs the 2e-2 gate)
and widened host-side.

All DMAs ride a single HWDGE queue (SP): both HWDGE queues share the
same 16 SDMA engines, so a second queue adds no bandwidth.  The unused
SWDGE (qPoolDynamic) and ACT queue declarations are stripped from the
module before compile (fewer declared rings; the runtime's fixed ~7.3us
semaphore-clear postamble is unaffected by this, measured).

A run of dummy matmuls (zero scratch, overwritten by the first conv
accumulation via start=True) warms the PE HAM clock gate while the
DMAs land.
"""

import os

import ml_dtypes
import numpy as np

try:
    import concourse.bass as bass
except ImportError:  # fall back to the repo checkout
    import sys
    sys.path.insert(0, "/opt/trn_rl_repo")
    import concourse.bass as bass
import concourse.mybir as mybir
import concourse.tile as tile
from concourse import bacc
from concourse.bass_utils import run_bass_kernel_spmd

F32 = mybir.dt.float32
F32R = mybir.dt.float32r
BF16 = mybir.dt.bfloat16
N_CORES = 8
ROWS_PER_CORE = 8          # LR rows per core
BAND_ROWS = ROWS_PER_CORE + 2
NPOS = ROWS_PER_CORE * 64  # 512 LR positions per core

# Taps t = ki*3+kj have band shift ki*66+kj.  band1 (in blob_a) holds the
# zero-padded band twice: p0-63 = band@+1 (a chunk at rhs offset roff
# sees shift roff-1), p64-127 = band@0 (shift roff).  K=128 chunks pair
# the delta-1 taps; taps 2/3/8 run K=64 on the p0-63 half (shift roff-1).
#   (band_tensor_idx, rhs_offset, K, taps, p_base)
ORDER = [
    (0, 1, 128, (0, 1), 0),     # shifts 0,1
    (0, 68, 128, (4, 5), 0),    # shifts 67,68
    (0, 133, 128, (6, 7), 0),   # shifts 132,133
    (0, 3, 64, (2,), 0),        # shift 2
    (0, 67, 64, (3,), 0),       # shift 66
    (0, 135, 64, (8,), 0),      # shift 134
]
COLS_B1 = 663  # 661 + pad cols so the tap-8 chunk's AP (135 + 8*66) fits
COLS_B2 = 528  # unused (kept for reference)
COLS_W = 6 * 48
COLS_A = COLS_B1 + COLS_W  # band ++ W in one blob -> one fat DMA

N_WARMUP_MM = 5

USE_BF16 = os.environ.get("METASR_DTYPE", "bf16") == "bf16"
QUEUE_MODE = os.environ.get("METASR_QUEUES", "sp16")

_CACHE = {}


def _build_program(use_bf16, queue_mode, early_dma=False, trim_end=True):
    """Build + compile the single-core Bass program (same for all cores)."""
    nc = bacc.Bacc("TRN2", target_bir_lowering=False, debug=False)

    dt = BF16 if use_bf16 else F32R
    odt = BF16 if use_bf16 else F32
    blob_a_d = nc.dram_tensor("blob_a", [128, COLS_A], dt, kind="ExternalInput")
    out48 = nc.dram_tensor("out48", [128, NPOS // 2], odt, kind="ExternalOutput")

    single_q = queue_mode == "sp16"

    with tile.TileContext(nc) as tc:
        with (
            tc.tile_pool(name="blobs", bufs=1) as blobs,
            tc.tile_pool(name="work", bufs=1) as work,
            tc.tile_pool(name="opool", bufs=1) as opool,
            tc.tile_pool(name="ps_rgb", bufs=1, space="PSUM") as ps_rgb,
        ):
            blob_a = blobs.tile([128, COLS_A], dt, tag="blob_a")
            nc.sync.dma_start(blob_a[:, :], blob_a_d[:, :])
            band1 = blob_a[:, 0:COLS_B1]
            wtile = blob_a[:, COLS_B1:COLS_A]

            # The conv output lives as [128, 256]: band rows 0-3 (256
            # positions) land in PSUM partitions 0-47, rows 4-7 in 64-111
            # (PSUM matmul base must be 0/32/64; rows 48-63/112-127 are
            # junk the host ignores).  Each chunk issues two 256-col
            # matmuls sharing one stationary load; the wider output
            # doubles the cast's DVE lanes and spreads the output DMA
            # across all 16 SDMA engines.
            hp = NPOS // 2
            rgb_ps = ps_rgb.tile([128, hp], F32, tag="rgb")
            warm = work.tile([128, NPOS], BF16, tag="warm")
            nc.vector.memset(warm[:, :], 0.0)
            nc.vector.memset(rgb_ps[:, :], 0.0)
            for i in range(N_WARMUP_MM):
                pbase = 64 * (i % 2)
                nc.tensor.matmul(
                    rgb_ps[pbase:pbase + 48, :], warm[:, 0:48], warm[:, 0:hp],
                    start=True, stop=True,
                )

            bands = [band1]
            last = len(ORDER) - 1
            for m, (bidx, roff, K, _taps, pb) in enumerate(ORDER):
                for s in range(2):
                    ro = roff + s * 4 * 66
                    rhs = bands[bidx][pb:pb + K, ro:ro + 4 * 66].rearrange(
                        "p (r c) -> p r c", c=66
                    )[:, :, 0:64]
                    nc.tensor.matmul(
                        rgb_ps[s * 64:s * 64 + 48, :],
                        wtile[pb:pb + K, m * 48:(m + 1) * 48], rhs,
                        start=(m == 0), stop=(m == last),
                    )

            # ---- write out: one cast copy + one DMA ----
            out_sb = opool.tile([128, hp], odt, tag="out")
            nc.vector.tensor_copy(out_sb[:, :], rgb_ps[:, :])
            nc.sync.dma_start(out48[:, :], out_sb[:, :])

    if single_q:
        # Both HWDGE queues share the 16 SDMA engines, and the runtime's
        # fixed postamble (semaphore-clear sweep) scales with declared
        # queue rings — keep only the SP HWDGE queue actually used.
        used = {"qSPDynamicHW"}
        nc.m.queues = [q for q in nc.m.queues if q.name in used]

    if early_dma:
        # Hoisting the blob_a DMA issue ahead of the framework's const-AP
        # memsets + barrier was tried and measured SLOWER (~16.7us vs
        # ~15.9us median): the transfer drains slower when it overlaps the
        # other cores' preamble phase, outweighing the ~0.7us earlier
        # issue.  Kept as a flag for reference; default off.
        main_blk, tile_blk = nc.m.functions[0].blocks[0], nc.m.functions[0].blocks[1]
        dma = next(i for i in tile_blk.instructions
                   if type(i).__name__ == "InstDMACopy")
        assert not (dma.sync_info and dma.sync_info.on_wait)
        tile_blk.instructions.remove(dma)
        main_blk.instructions.insert(1, dma)

    if trim_end:
        # The TileContext epilogue ends with TWO all-engine barriers; the
        # second is redundant (every semaphore is stable after the first,
        # which already follows SP's DMA-drain waits and the tile
        # RANGE_CLEAR).  Dropping it lets each engine flow into the
        # runtime's fixed semaphore-clear postamble one barrier earlier.
        eb = nc.m.functions[0].blocks[-1]
        names = [type(i).__name__ for i in eb.instructions]
        isa_idx = names.index("InstISA")  # the tile RANGE_CLEAR on Pool
        assert isa_idx > len(names) // 3, (names, isa_idx)
        del eb.instructions[isa_idx + 1:]

        # Letting PE and Activation ALSO skip this barrier (so their
        # postamble semaphore-clear slices overlap the conv/DMAs) was
        # tried and measured SLOWER (17.6 vs 15.7us): clearing the
        # walrus-range sems 7-104 while DMAs are in flight throttles the
        # queue machinery.  Keep all engines behind the DMA drain.

    nc.compile()
    return nc


def _round_f32r(x):
    """Round fp32 to the fp32r-representable set (bf16 hi + bf16 lo pair)."""
    hi = x.astype(ml_dtypes.bfloat16).astype(np.float32)
    lo = (x - hi).astype(ml_dtypes.bfloat16).astype(np.float32)
    return hi + lo


def _host_prep(feat, w1, b1, w2, b2, use_bf16):
    """Compute the 16-phase conv weights and pack per-core band blobs."""
    feat = np.ascontiguousarray(np.asarray(feat, dtype=np.float32))[0]  # [64,64,64]
    w1 = np.asarray(w1, dtype=np.float32)
    b1 = np.asarray(b1, dtype=np.float32)
    w2 = np.asarray(w2, dtype=np.float32)
    b2 = np.asarray(b2, dtype=np.float32)

    dydx = np.arange(16)
    mlpin = np.stack(
        [dydx // 4 / 4.0, dydx % 4 / 4.0, np.full(16, 0.25)], axis=1
    ).astype(np.float32)  # [16, 3]
    h = np.maximum(mlpin @ w1 + b1, 0.0).astype(np.float32)      # [16, 256]
    pw = (h @ w2 + b2).astype(np.float32).reshape(16, 64, 9, 3)  # [ph, c, t, o]

    wblob = np.zeros((128, COLS_W), dtype=np.float32)
    for m, (_bidx, _roff, _K, taps, pb) in enumerate(ORDER):
        for slot, t in enumerate(taps):
            # rows pb + slot*64 + c ; cols m*48 + o*16 + ph
            r0 = pb + slot * 64
            wblob[r0:r0 + 64, m * 48:(m + 1) * 48] = \
                pw[:, :, t, :].transpose(1, 2, 0).reshape(64, 48)

    featp = np.zeros((64, 66, 66), dtype=np.float32)
    featp[:, 1:65, 1:65] = feat

    if use_bf16:
        wblob = wblob.astype(ml_dtypes.bfloat16)
        featp = featp.astype(ml_dtypes.bfloat16)
    else:
        wblob = _round_f32r(wblob)
        featp = _round_f32r(featp)
    ndt = featp.dtype

    blobs_a = []
    for core in range(N_CORES):
        r0 = core * ROWS_PER_CORE
        band = featp[:, r0:r0 + BAND_ROWS, :].reshape(64, BAND_ROWS * 66)
        ab = np.zeros((128, COLS_A), dtype=ndt)
        ab[0:64, 1:661] = band
        ab[64:128, 0:660] = band
        ab[:, COLS_B1:COLS_A] = wblob
        blobs_a.append(ab)
    return blobs_a


def _assemble(per_core_out48):
    """[8 x [128, 256]] -> [1, 3, 256, 256].

    Rows 0-47 / 64-111 of a core's block hold band rows 0-3 / 4-7
    (rows 48-63 and 112-127 are junk).  Within a half, row j is
    (o = j//16, dy = (j%16)//4, dx = j%4); col is (r4, x); the LR row
    is core*8 + half*4 + r4.
    """
    full = np.stack([np.asarray(o, dtype=np.float32) for o in per_core_out48])
    full = full.reshape(8, 2, 64, 4, 64)[:, :, 0:48]  # [core, half, 48, r4, x]
    full = full.reshape(8, 2, 3, 4, 4, 4, 64)  # [core, half, o, dy, dx, r4, x]
    rgb = full.transpose(2, 0, 1, 5, 3, 6, 4).reshape(3, 256, 256)
    return np.ascontiguousarray(rgb)[None]


def get_program():
    key = ("nc", USE_BF16, QUEUE_MODE)
    if key not in _CACHE:
        _CACHE[key] = _build_program(USE_BF16, QUEUE_MODE)
    return _CACHE[key]


def run(feat, w1, b1, w2, b2, out_h, out_w, trace=False, **spmd_kwargs):
    assert int(out_h) == 256 and int(out_w) == 256
    nc = get_program()
    blobs_a = _host_prep(feat, w1, b1, w2, b2, USE_BF16)
    in_maps = [{"blob_a": blobs_a[core]} for core in range(N_CORES)]
    res = run_bass_kernel_spmd(
        nc, in_maps, core_ids=list(range(N_CORES)), trace=trace, **spmd_kwargs
    )
    out = _assemble([res.results[core]["out48"] for core in range(N_CORES)])
    return out, res


def kernel(feat, w1, b1, w2, b2, out_h, out_w):
    out, _ = run(feat, w1, b1, w2, b2, out_h, out_w, trace=False)
    return out


# revision 64
# speedup vs baseline: 1.0682x; 1.0358x over previous
"""MetaSR super-resolution Trainium2 kernel.

Structure exploited: out_h=out_w=256 with H=W=64 LR grid means the scale
factor is exactly 4, so the nearest-neighbor gather index is iy=oy//4,
ix=ox//4 and the per-query MLP input collapses to 16 distinct subpixel
phases [dy/4, dx/4, 0.25].  The whole model becomes a 3x3 conv with 64
input / 48 output channels (3 RGB x 16 phases) + pixel shuffle, whose
48x576 weight predw = relu([16,3] @ w1 + b1) @ w2 + b2 is a tiny
16-phase MLP evaluated host-side (14 MFLOP of the model's 240 MFLOP;
the 226 MFLOP conv runs on device).

Sharding: data-parallel over LR rows (8 rows per core, 10-row halo band),
conv weights replicated.

The conv contraction (K = 9 taps x 64 ch = 576) is chunked K=128 by
pairing taps.  Each core holds the zero-padded band twice in a
128-partition tile at free-dim offsets that differ by the two taps'
shift delta, so one K=128 matmul consumes two taps without
materializing the unfolded tensor:
  band free index = r*66 + x  (66-wide zero-padded rows), tap (ki,kj)
  shift = ki*66 + kj.
Three K=128 chunks pair the delta-1 taps; the three leftover taps run
as K=64 chunks on the band@+1 half.  Everything (band + weights) rides
ONE fat DMA (1902B rows, few descriptors) — a single semaphore gates
the whole conv.

Each chunk issues TWO 256-col matmuls (band rows 0-3 -> PSUM
partitions 0-47, rows 4-7 -> 64-111) sharing one stationary load: the
two PE output row groups (h0/h64) dual-issue, running the pair
concurrently — the conv takes ~1.5us vs ~2.5us for 512-col single
matmuls.  The wider [128, 256] result also doubles the output cast's
DVE lanes and spreads the output DMA across all 16 SDMA engines.

Band and weights are bf16 (PSUM accumulates fp32): halves DMA traffic.
The output is written back as bf16 (rel err ~3.3e-3 vs the 2e-2 gate)
and widened host-side.

All DMAs ride a single HWDGE queue (SP): both HWDGE queues share the
same 16 SDMA engines, so a second queue adds no bandwidth.  The unused
SWDGE (qPoolDynamic) and ACT queue declarations are stripped from the
module before compile (fewer declared rings; the runtime's fixed ~7.3us
semaphore-clear postamble is unaffected by this, measured).

A run of dummy matmuls (zero scratch, overwritten by the first conv
accumulation via start=True) warms the PE HAM clock gate while the
DMAs land.
"""

import os

import ml_dtypes
import numpy as np

try:
    import concourse.bass as bass
except ImportError:  # fall back to the repo checkout
    import sys
    sys.path.insert(0, "/opt/trn_rl_repo")
    import concourse.bass as bass
import concourse.mybir as mybir
import concourse.tile as tile
from concourse import bacc
from concourse.bass_utils import run_bass_kernel_spmd

F32 = mybir.dt.float32
F32R = mybir.dt.float32r
BF16 = mybir.dt.bfloat16
N_CORES = 8
ROWS_PER_CORE = 8          # LR rows per core
BAND_ROWS = ROWS_PER_CORE + 2
NPOS = ROWS_PER_CORE * 64  # 512 LR positions per core

# Taps t = ki*3+kj have band shift ki*66+kj.  band1 (in blob_a) holds the
# zero-padded band twice: p0-63 = band@+1 (a chunk at rhs offset roff
# sees shift roff-1), p64-127 = band@0 (shift roff).  K=128 chunks pair
# the delta-1 taps; taps 2/3/8 run K=64 on the p0-63 half (shift roff-1).
#   (band_tensor_idx, rhs_offset, K, taps, p_base)
# (rhs_offset, K, taps, p_base, w_col).  NOTE: sharing one W column
# block between taps 2 and 3 by running t3 at partition base 64 crashes
# NRT (INTERNAL) — pb=64 rhs only works at large rhs offsets (134 ok,
# 2 and 66 crash).  All chunks stay on the p0-63 half.
ORDER = [
    (1, 128, (0, 1), 0, 0),      # shifts 0,1
    (68, 128, (4, 5), 0, 48),    # shifts 67,68
    (133, 128, (6, 7), 0, 96),   # shifts 132,133
    (3, 64, (2,), 0, 144),       # shift 2
    (67, 64, (3,), 0, 192),      # shift 66
    (135, 64, (8,), 0, 240),     # shift 134
]
COLS_B1 = 663  # 661 + pad cols so the tap-8 chunk's AP (135 + 8*66) fits
COLS_W = 6 * 48
COLS_A = COLS_B1 + COLS_W  # band ++ W in one blob -> one fat DMA

N_WARMUP_MM = 5

USE_BF16 = os.environ.get("METASR_DTYPE", "bf16") == "bf16"
QUEUE_MODE = os.environ.get("METASR_QUEUES", "sp16")

_CACHE = {}


def _build_program(use_bf16, queue_mode, early_dma=False, trim_end=True):
    """Build + compile the single-core Bass program (same for all cores)."""
    nc = bacc.Bacc("TRN2", target_bir_lowering=False, debug=False)

    dt = BF16 if use_bf16 else F32R
    odt = BF16 if use_bf16 else F32
    blob_a_d = nc.dram_tensor("blob_a", [128, COLS_A], dt, kind="ExternalInput")
    out48 = nc.dram_tensor("out48", [128, NPOS // 2], odt, kind="ExternalOutput")

    single_q = queue_mode == "sp16"

    with tile.TileContext(nc) as tc:
        with (
            tc.tile_pool(name="blobs", bufs=1) as blobs,
            tc.tile_pool(name="work", bufs=1) as work,
            tc.tile_pool(name="opool", bufs=1) as opool,
            tc.tile_pool(name="ps_rgb", bufs=1, space="PSUM") as ps_rgb,
        ):
            blob_a = blobs.tile([128, COLS_A], dt, tag="blob_a")
            nc.sync.dma_start(blob_a[:, :], blob_a_d[:, :])
            band1 = blob_a[:, 0:COLS_B1]
            wtile = blob_a[:, COLS_B1:COLS_A]

            # The conv output lives as [128, 256]: band rows 0-3 (256
            # positions) land in PSUM partitions 0-47, rows 4-7 in 64-111
            # (PSUM matmul base must be 0/32/64; rows 48-63/112-127 are
            # junk the host ignores).  Each chunk issues two 256-col
            # matmuls sharing one stationary load; the wider output
            # doubles the cast's DVE lanes and spreads the output DMA
            # across all 16 SDMA engines.
            hp = NPOS // 2
            rgb_ps = ps_rgb.tile([128, hp], F32, tag="rgb")
            warm = work.tile([128, NPOS], BF16, tag="warm")
            nc.vector.memset(warm[:, :], 0.0)
            nc.vector.memset(rgb_ps[:, :], 0.0)
            for i in range(N_WARMUP_MM):
                pbase = 64 * (i % 2)
                nc.tensor.matmul(
                    rgb_ps[pbase:pbase + 48, :], warm[:, 0:48], warm[:, 0:hp],
                    start=True, stop=True,
                )

            last = len(ORDER) - 1
            for m, (roff, K, _taps, pb, wc) in enumerate(ORDER):
                for s in range(2):
                    ro = roff + s * 4 * 66
                    rhs = band1[pb:pb + K, ro:ro + 4 * 66].rearrange(
                        "p (r c) -> p r c", c=66
                    )[:, :, 0:64]
                    nc.tensor.matmul(
                        rgb_ps[s * 64:s * 64 + 48, :],
                        wtile[pb:pb + K, wc:wc + 48], rhs,
                        start=(m == 0), stop=(m == last),
                    )

            # ---- write out: one cast copy + one DMA ----
            out_sb = opool.tile([128, hp], odt, tag="out")
            nc.vector.tensor_copy(out_sb[:, :], rgb_ps[:, :])
            nc.sync.dma_start(out48[:, :], out_sb[:, :])

    if single_q:
        # Both HWDGE queues share the 16 SDMA engines, and the runtime's
        # fixed postamble (semaphore-clear sweep) scales with declared
        # queue rings — keep only the SP HWDGE queue actually used.
        used = {"qSPDynamicHW"}
        nc.m.queues = [q for q in nc.m.queues if q.name in used]

    if early_dma:
        # Hoisting the blob_a DMA issue ahead of the framework's const-AP
        # memsets + barrier was tried and measured SLOWER (~16.7us vs
        # ~15.9us median): the transfer drains slower when it overlaps the
        # other cores' preamble phase, outweighing the ~0.7us earlier
        # issue.  Kept as a flag for reference; default off.
        main_blk, tile_blk = nc.m.functions[0].blocks[0], nc.m.functions[0].blocks[1]
        dma = next(i for i in tile_blk.instructions
                   if type(i).__name__ == "InstDMACopy")
        assert not (dma.sync_info and dma.sync_info.on_wait)
        tile_blk.instructions.remove(dma)
        main_blk.instructions.insert(1, dma)

    if trim_end:
        # The TileContext epilogue ends with TWO all-engine barriers; the
        # second is redundant (every semaphore is stable after the first,
        # which already follows SP's DMA-drain waits and the tile
        # RANGE_CLEAR).  Dropping it lets each engine flow into the
        # runtime's fixed semaphore-clear postamble one barrier earlier.
        eb = nc.m.functions[0].blocks[-1]
        names = [type(i).__name__ for i in eb.instructions]
        isa_idx = names.index("InstISA")  # the tile RANGE_CLEAR on Pool
        assert isa_idx > len(names) // 3, (names, isa_idx)
        del eb.instructions[isa_idx + 1:]

        # Letting PE and Activation ALSO skip this barrier (so their
        # postamble semaphore-clear slices overlap the conv/DMAs) was
        # tried and measured SLOWER (17.6 vs 15.7us): clearing the
        # walrus-range sems 7-104 while DMAs are in flight throttles the
        # queue machinery.  Keep all engines behind the DMA drain.

    nc.compile()
    return nc


def _round_f32r(x):
    """Round fp32 to the fp32r-representable set (bf16 hi + bf16 lo pair)."""
    hi = x.astype(ml_dtypes.bfloat16).astype(np.float32)
    lo = (x - hi).astype(ml_dtypes.bfloat16).astype(np.float32)
    return hi + lo


def _host_prep(feat, w1, b1, w2, b2, use_bf16):
    """Compute the 16-phase conv weights and pack per-core band blobs."""
    feat = np.ascontiguousarray(np.asarray(feat, dtype=np.float32))[0]  # [64,64,64]
    w1 = np.asarray(w1, dtype=np.float32)
    b1 = np.asarray(b1, dtype=np.float32)
    w2 = np.asarray(w2, dtype=np.float32)
    b2 = np.asarray(b2, dtype=np.float32)

    dydx = np.arange(16)
    mlpin = np.stack(
        [dydx // 4 / 4.0, dydx % 4 / 4.0, np.full(16, 0.25)], axis=1
    ).astype(np.float32)  # [16, 3]
    h = np.maximum(mlpin @ w1 + b1, 0.0).astype(np.float32)      # [16, 256]
    pw = (h @ w2 + b2).astype(np.float32).reshape(16, 64, 9, 3)  # [ph, c, t, o]

    wblob = np.zeros((128, COLS_W), dtype=np.float32)
    for (_roff, _K, taps, pb, wc) in ORDER:
        for slot, t in enumerate(taps):
            # rows pb + slot*64 + c ; cols wc + o*16 + ph
            r0 = pb + slot * 64
            wblob[r0:r0 + 64, wc:wc + 48] = \
                pw[:, :, t, :].transpose(1, 2, 0).reshape(64, 48)

    featp = np.zeros((64, 66, 66), dtype=np.float32)
    featp[:, 1:65, 1:65] = feat

    if use_bf16:
        wblob = wblob.astype(ml_dtypes.bfloat16)
        featp = featp.astype(ml_dtypes.bfloat16)
    else:
        wblob = _round_f32r(wblob)
        featp = _round_f32r(featp)
    ndt = featp.dtype

    blobs_a = []
    for core in range(N_CORES):
        r0 = core * ROWS_PER_CORE
        band = featp[:, r0:r0 + BAND_ROWS, :].reshape(64, BAND_ROWS * 66)
        ab = np.zeros((128, COLS_A), dtype=ndt)
        ab[0:64, 1:661] = band
        ab[64:128, 0:660] = band
        ab[:, COLS_B1:COLS_A] = wblob
        blobs_a.append(ab)
    return blobs_a


def _assemble(per_core_out48):
    """[8 x [128, 256]] -> [1, 3, 256, 256].

    Rows 0-47 / 64-111 of a core's block hold band rows 0-3 / 4-7
    (rows 48-63 and 112-127 are junk).  Within a half, row j is
    (o = j//16, dy = (j%16)//4, dx = j%4); col is (r4, x); the LR row
    is core*8 + half*4 + r4.
    """
    full = np.stack([np.asarray(o, dtype=np.float32) for o in per_core_out48])
    full = full.reshape(8, 2, 64, 4, 64)[:, :, 0:48]  # [core, half, 48, r4, x]
    full = full.reshape(8, 2, 3, 4, 4, 4, 64)  # [core, half, o, dy, dx, r4, x]
    rgb = full.transpose(2, 0, 1, 5, 3, 6, 4).reshape(3, 256, 256)
    return np.ascontiguousarray(rgb)[None]


def get_program():
    key = ("nc", USE_BF16, QUEUE_MODE)
    if key not in _CACHE:
        _CACHE[key] = _build_program(USE_BF16, QUEUE_MODE)
    return _CACHE[key]


def run(feat, w1, b1, w2, b2, out_h, out_w, trace=False, **spmd_kwargs):
    assert int(out_h) == 256 and int(out_w) == 256
    nc = get_program()
    blobs_a = _host_prep(feat, w1, b1, w2, b2, USE_BF16)
    in_maps = [{"blob_a": blobs_a[core]} for core in range(N_CORES)]
    res = run_bass_kernel_spmd(
        nc, in_maps, core_ids=list(range(N_CORES)), trace=trace, **spmd_kwargs
    )
    out = _assemble([res.results[core]["out48"] for core in range(N_CORES)])
    return out, res


def kernel(feat, w1, b1, w2, b2, out_h, out_w):
    out, _ = run(feat, w1, b1, w2, b2, out_h, out_w, trace=False)
    return out
